# revision 1
# baseline (speedup 1.0000x reference)
"""Attention graph convolution (GAT layer) on 8 TRN2 NeuronCores.

Reference computation (all fp32):
    h   = input @ W                      # (N, 64)
    e   = leakyrelu(h@a1 + (h@a2).T)     # (N, N)
    att = softmax(where(adj>0, e, -inf)) # row softmax
    out = elu(att @ h)                   # (N, 64)

Sharding: rows of e/att (= output rows) are split across 8 cores,
1536 rows each.  h (N x 64) is computed on every core (tiny).

Per-core algorithm (core owns rows I, |I| = 1536):
  - no max-subtraction softmax: z values are small (|z| < ~30), so
    U[i,j] = adj[i,j] * exp(leakyrelu(Wh1_i + Wh2_j)) cannot overflow and
    equals the reference numerator up to the common exp(-max) factor.
  - denominator via ones-column: P = U @ [h | 1]; out = elu(P[:, :64] / P[:, 64])
  - U is built in TRANSPOSED layout [j partitions, i free] so it can feed
    the PE matmul (contraction dim = partition dim) with no U transpose:
        P.T[f, i] = sum_j h_ext[j, f] * U.T[j, i]
    adj row-blocks are DMA'd contiguously (int32 -> bf16 cast in SWDGE,
    exact for 0/1) and transposed 128x128-at-a-time on the tensor engine
    into PSUM; the mask multiply reads adj.T directly from PSUM.
  - h/Wh1/Wh2 production (phase 1) is interleaved with the first window
    of the main loop so it overlaps the adjacency DMA stream.
"""

import numpy as np

N_TOTAL = 12288
K_IN = 128
F_OUT = 64
N_CORES = 8
ALPHA = 0.2


def build_program(
    nt: int,          # total nodes (columns of adj)
    no: int,          # nodes owned by this core (rows of adj block)
    jw: int,          # j window size (columns resident in SBUF at once)
    u_bf16: bool = False,  # U / h_ext in bf16 for the big matmul
    lrelu_act_frac: float = 0.70,  # j-chunk fraction with leakyrelu on ACT
):
    from contextlib import ExitStack

    import concourse.bass as bass
    import concourse.mybir as mybir
    import concourse.tile as tile
    from concourse import bacc
    from concourse.alu_op_type import AluOpType
    from concourse.masks import make_identity

    f32 = mybir.dt.float32
    i32 = mybir.dt.int32
    bf16 = mybir.dt.bfloat16
    AF = mybir.ActivationFunctionType
    u_dt = bf16 if u_bf16 else f32

    P = 128
    F = F_OUT
    FE = F + 1                    # h columns + ones column
    K = K_IN
    assert nt % P == 0 and no % P == 0 and jw % P == 0 and nt % jw == 0
    ncj = nt // P                 # global j chunks
    nw = nt // jw                 # windows
    cpw = jw // P                 # j chunks per window
    nic = no // P                 # i chunks (own rows)
    S = 512                       # i split for matmul N-dim / psum banks
    ns = (no + S - 1) // S
    assert no % S == 0 or ns == 1

    nc = bacc.Bacc("TRN2", target_bir_lowering=False, debug=False,
                   num_devices=1)

    inp = nc.dram_tensor("input", [nt, K], f32, kind="ExternalInput")
    inp_own = nc.dram_tensor("input_own", [no, K], f32, kind="ExternalInput")
    adj_own = nc.dram_tensor("adj_own", [no, nt], i32, kind="ExternalInput")
    w_d = nc.dram_tensor("W", [K, F], f32, kind="ExternalInput")
    a_d = nc.dram_tensor("a", [2 * F, 1], f32, kind="ExternalInput")
    out_d = nc.dram_tensor("out", [no, F], f32, kind="ExternalOutput")

    with tile.TileContext(nc) as tc, ExitStack() as ctx:
        consts = ctx.enter_context(tc.tile_pool(name="consts", bufs=1))

        identity = consts.tile([P, P], f32)
        make_identity(nc, identity)
        identity_bf = consts.tile([P, P], bf16)
        nc.vector.tensor_copy(identity_bf[:], identity[:])

        # shared small-psum scratch (phases 0/1/3); 1 bank — PSUM budget is
        # 4 (adjT double-buffered) + 3 (P.T accumulator) + 1 = 8 banks.
        scr_ps = ctx.enter_context(
            tc.tile_pool(name="scr_ps", bufs=1, space="PSUM"))

        # ---- phase 0: Wa1 = W @ a1, Wa2 = W @ a2 -------------------------
        # (no strided DRAM reads: a 4-byte-element transposed W read costs
        # ~71 us of HWDGE descriptor generation and stalls the Sync queue)
        wwa2_sb = consts.tile([K, FE], f32)    # [W | Wa2] (128 x 65)
        nc.sync.dma_start(wwa2_sb[:, 0:F], w_d.ap())
        a_row = consts.tile([1, 2 * F], f32)   # a as a single-partition row
        nc.sync.dma_start(a_row[:], a_d.ap().rearrange("n o -> o n"))

        ones_sb = consts.tile([P, P], f32)
        nc.vector.memset(ones_sb[:], 1.0)
        # replicate a across partitions via a K=1 matmul with a ones row
        a_rep = consts.tile([P, 2 * F], f32)
        a_rep_ps = scr_ps.tile([P, 2 * F], f32, tag="scr")
        nc.tensor.matmul(a_rep_ps[:], ones_sb[0:1, :], a_row[:],
                         start=True, stop=True)
        nc.vector.tensor_copy(a_rep[:], a_rep_ps[:])

        wa12_sb = consts.tile([K, 2], f32)
        wtmp = consts.tile([K, F], f32)
        nc.vector.tensor_tensor(wtmp[:], wwa2_sb[:, 0:F], a_rep[:, 0:F],
                                AluOpType.mult)
        nc.vector.tensor_reduce(wa12_sb[:, 0:1], wtmp[:],
                                mybir.AxisListType.X, AluOpType.add)
        nc.vector.tensor_tensor(wtmp[:], wwa2_sb[:, 0:F], a_rep[:, F:2 * F],
                                AluOpType.mult)
        nc.vector.tensor_reduce(wa12_sb[:, 1:2], wtmp[:],
                                mybir.AxisListType.X, AluOpType.add)
        nc.vector.tensor_copy(wwa2_sb[:, F:FE], wa12_sb[:, 1:2])
        wa1_rep = consts.tile([K, P], f32)     # Wa1 replicated to 128 cols
        nc.vector.tensor_scalar(wa1_rep[:], ones_sb[:], wa12_sb[:, 0:1], None,
                                AluOpType.mult)

        # ---- phase 1a: Wh1_rep[p, x] = Wh1[own x] for all p --------------
        # Wh1_rep = wa1_rep.T @ input_own.T ; input_own.T via PE transposes.
        wh1_rep = consts.tile([P, no], f32)
        into_sb = consts.tile([K, no], f32)    # input_own.T
        in_t = ctx.enter_context(tc.tile_pool(name="in_t", bufs=4))
        for ic in range(nic):
            ich = in_t.tile([P, K], f32, tag="ich")
            nc.sync.dma_start(ich[:], inp_own[ic * P:(ic + 1) * P, :])
            itp = scr_ps.tile([K, P], f32, tag="scr")
            nc.tensor.transpose(itp[:], ich[:], identity[:])
            nc.vector.tensor_copy(into_sb[:, ic * P:(ic + 1) * P], itp[:])
        for s in range(ns):
            sw = min(S, no - s * S)
            w1p = scr_ps.tile([P, S], f32, tag="scr")
            nc.tensor.matmul(w1p[:, 0:sw], wa1_rep[:],
                             into_sb[:, s * S:s * S + sw],
                             start=True, stop=True)
            nc.vector.tensor_copy(wh1_rep[:, s * S:s * S + sw], w1p[:, 0:sw])

        # ---- phase 1b (emitted interleaved below): h_ext, Wh2 ------------
        h_ext = consts.tile([P, ncj, FE], u_dt)
        wh2_sb = consts.tile([P, ncj], f32)
        nc.vector.memset(h_ext[:, :, F], 1.0)

        def phase1b_chunk(jc):
            # input[jc].T via PE transpose; h_ext[:, jc, :] = [h | Wh2-col]
            jch = in_t.tile([P, K], f32, tag="ich")
            nc.sync.dma_start(jch[:], inp[jc * P:(jc + 1) * P, :])
            jtp = scr_ps.tile([K, P], f32, tag="scr")
            nc.tensor.transpose(jtp[:], jch[:], identity[:])
            jts = in_t.tile([K, P], f32, tag="jts")
            nc.vector.tensor_copy(jts[:], jtp[:])
            hw_ps = scr_ps.tile([P, FE], f32, tag="scr")
            nc.tensor.matmul(hw_ps[:], jts[:], wwa2_sb[:],
                             start=True, stop=True)
            nc.scalar.copy(h_ext[:, jc, 0:F], hw_ps[:, 0:F])
            nc.vector.tensor_copy(wh2_sb[:, jc:jc + 1], hw_ps[:, F:FE])

        # ---- phase 2: main loop over j windows / j chunks ----------------
        pt_pool = ctx.enter_context(
            tc.tile_pool(name="pt_acc", bufs=1, space="PSUM"))
        pt_ps = pt_pool.tile([FE, no], f32)

        n_act = int(round(lrelu_act_frac * ncj))

        def lrelu_engine(jc):
            # deterministic interleave of ACT / DVE chunks
            return "act" if (jc * 7919) % ncj < n_act else "dve"

        def lrelu_chunk(jc, dst):
            if lrelu_engine(jc) == "act":
                nc.scalar.activation(dst, wh1_rep[:], AF.Prelu,
                                     bias=wh2_sb[:, jc:jc + 1],
                                     scale=1.0, alpha=ALPHA)
            else:
                # t = 0.2 * (Wh1 + Wh2) ; E = max(Wh1 + Wh2, t)
                nc.vector.tensor_scalar(dst, wh1_rep[:],
                                        wh2_sb[:, jc:jc + 1], ALPHA,
                                        AluOpType.add, AluOpType.mult)
                nc.vector.scalar_tensor_tensor(
                    dst, wh1_rep[:], wh2_sb[:, jc:jc + 1], dst,
                    AluOpType.add, AluOpType.max)

        # phase-1b chunks are produced spread over the first nw-1 windows,
        # always ahead of their first use by the main loop.
        per_w = -(-ncj // max(1, nw - 1))
        npair = cpw // 2
        assert cpw % 2 == 0

        with (
            tc.tile_pool(name="adjw", bufs=2 * nic) as adjw_pool,
            tc.tile_pool(name="adjt", bufs=4, space="PSUM") as tr_pool,
            tc.tile_pool(name="epool", bufs=2) as e_pool,
            tc.tile_pool(name="upool", bufs=2 * ns) as u_pool,
        ):
            adjw = {}
            for w in range(nw):
                # adj window DMA (SWDGE cast int32 -> bf16), one per i chunk
                for ic in range(nic):
                    t = adjw_pool.tile([P, jw], bf16, tag="adjw",
                                       name=f"adjw_{w}_{ic}")
                    nc.gpsimd.dma_start(
                        t[:],
                        adj_own[ic * P:(ic + 1) * P, w * jw:(w + 1) * jw])
                    adjw[ic] = t
                wchunks = list(range(w * per_w, min(ncj, (w + 1) * per_w)))
                for jp in range(npair):
                    for k, jc1b in enumerate(wchunks):
                        if k * npair // len(wchunks) == jp:
                            phase1b_chunk(jc1b)
                    jcs = [w * cpw + 2 * jp, w * cpw + 2 * jp + 1]
                    # leakyrelu for both chunks, one batched exp
                    e_sb = e_pool.tile([P, 2, no], f32, tag="e")
                    for q, jc in enumerate(jcs):
                        lrelu_chunk(jc, e_sb[:, q, :])
                    nc.scalar.activation(e_sb[:], e_sb[:], AF.Exp)
                    for q, jc in enumerate(jcs):
                        jcl = jc - w * cpw
                        for s in range(ns):
                            sw = min(S, no - s * S)
                            # adj.T for this i-split: regular bf16 matmuls
                            # against the identity (exact for 0/1) — gets
                            # FWL + weight double-buffering, unlike the
                            # transpose_mode path.
                            at_ps = tr_pool.tile([P, S], f32, tag="adjt")
                            for q2 in range(sw // P):
                                ic = (s * S) // P + q2
                                nc.tensor.matmul(
                                    at_ps[:, q2 * P:(q2 + 1) * P],
                                    adjw[ic][:, jcl * P:(jcl + 1) * P],
                                    identity_bf[:], start=True, stop=True)
                            # U = E * adj.T ; P.T += h_ext.T @ U
                            u_sb = u_pool.tile([P, S], u_dt, tag="u")
                            nc.vector.tensor_tensor(
                                u_sb[:, 0:sw],
                                e_sb[:, q, s * S:s * S + sw],
                                at_ps[:, 0:sw], AluOpType.mult)
                            nc.tensor.matmul(pt_ps[:, s * S:s * S + sw],
                                             h_ext[:, jc, :],
                                             u_sb[:, 0:sw],
                                             start=(jc == 0),
                                             stop=(jc == ncj - 1))

        # ---- phase 3: out = elu(P[:, :64] / P[:, 64]) --------------------
        pt_sb = consts.tile([FE, no], f32)
        nc.vector.tensor_copy(pt_sb[:], pt_ps[:])
        with tc.tile_pool(name="fin_sb", bufs=4) as fin_sb:
            for ic in range(nic):
                ptp = scr_ps.tile([P, FE], f32, tag="scr")
                nc.tensor.transpose(ptp[:], pt_sb[:, ic * P:(ic + 1) * P],
                                    identity[0:FE, 0:FE])
                rec = fin_sb.tile([P, 1], f32, tag="rec")
                nc.vector.reciprocal(rec[:], ptp[:, F:FE])
                hp = fin_sb.tile([P, F], f32, tag="hp")
                nc.vector.tensor_scalar(hp[:], ptp[:, 0:F], rec[:], None,
                                        AluOpType.mult)
                # elu(x) = max(x,0) + exp(min(x,0)) - 1
                mn = fin_sb.tile([P, F], f32, tag="mn")
                nc.vector.tensor_scalar(mn[:], hp[:], 0.0, None, AluOpType.min)
                nc.scalar.activation(mn[:], mn[:], AF.Exp)
                nc.vector.tensor_scalar(hp[:], hp[:], 0.0, None, AluOpType.max)
                ob = fin_sb.tile([P, F], f32, tag="ob")
                nc.vector.scalar_tensor_tensor(
                    ob[:], mn[:], 1.0, hp[:],
                    AluOpType.subtract, AluOpType.add)
                nc.sync.dma_start(out_d[ic * P:(ic + 1) * P, :], ob[:])

    nc.compile()
    return nc


_CACHE = {}


def _get_program(nt, no, jw, **kw):
    key = (nt, no, jw, tuple(sorted(kw.items())))
    if key not in _CACHE:
        _CACHE[key] = build_program(nt, no, jw, **kw)
    return _CACHE[key]


def kernel(input, adj, W, a):
    from concourse.bass_utils import run_bass_kernel_spmd

    input = np.ascontiguousarray(input, dtype=np.float32)
    adj = np.ascontiguousarray(adj, dtype=np.int32)
    W = np.ascontiguousarray(W, dtype=np.float32)
    a = np.ascontiguousarray(a, dtype=np.float32)

    nt = input.shape[0]
    no = nt // N_CORES
    nc = _get_program(nt, no, 2048)

    in_maps = []
    for c in range(N_CORES):
        in_maps.append({
            "input": input,
            "input_own": input[c * no:(c + 1) * no],
            "adj_own": adj[c * no:(c + 1) * no],
            "W": W,
            "a": a,
        })
    res = run_bass_kernel_spmd(nc, in_maps, list(range(N_CORES)))
    return np.concatenate([r["out"] for r in res.results], axis=0)



# revision 2
# speedup vs baseline: 1.0064x; 1.0064x over previous
"""Attention graph convolution (GAT layer) on 8 TRN2 NeuronCores.

Reference computation (all fp32):
    h   = input @ W                      # (N, 64)
    e   = leakyrelu(h@a1 + (h@a2).T)     # (N, N)
    att = softmax(where(adj>0, e, -inf)) # row softmax
    out = elu(att @ h)                   # (N, 64)

Sharding: rows of e/att (= output rows) are split across 8 cores,
1536 rows each.  h (N x 64) is computed on every core (tiny).
Each core's adjacency row-block is handed to it TRANSPOSED by the host
(adjt[j, i] = adj[own_i, j], layout-only prep, same byte count), so the
SWDGE cast DMA (int32 -> bf16, exact for 0/1) lands adj.T directly in
the [j partitions, i free] layout the PE contraction needs -- no
on-device transposes of the big matrix and no PSUM operands in the
element-wise path.

Per-core algorithm (core owns rows I, |I| = 1536):
  - no max-subtraction softmax: |z| < ~30 so U = adj.T * exp(lrelu(z))
    cannot overflow in f32/bf16 and equals the reference numerator.
  - denominator via ones-column: P.T = [h | 1].T @ U.T;
    out = elu(P[:, :64] / P[:, 64]).
  - j chunks are PERMUTED: chunk n holds nodes {96p + n : p in [0,128)}
    on partition p.  This makes the full-input load a single contiguous
    DMA ([128, 96*128] f32, 48KB descriptors) and keeps the adj.T window
    DMAs at 6KB/descriptor.  The accumulation sums over all j, so chunk
    order is irrelevant; own-row (i) indexing stays in natural order.
  - element-wise pipeline is all 2-byte SBUF tiles: leakyrelu on DVE
    (2 x tensor_scalar, 4x perf mode) or ACT (Prelu, schedule-balanced),
    exp on ACT (fp16 -> bf16), adjacency mask on DVE (bf16, 2x mode).
  - accumulation matmuls are bf16 (1 cycle/row vs fp32's 4).
"""

import numpy as np

N_TOTAL = 12288
K_IN = 128
F_OUT = 64
N_CORES = 8
ALPHA = 0.2


def build_program(
    nt: int,          # total nodes (rows of adjt)
    no: int,          # nodes owned by this core (cols of adjt)
    jw: int,          # j window size (adj.T rows resident in SBUF at once)
    act_frac: float = 0.25,  # fraction of j chunks with leakyrelu on ACT
):
    from contextlib import ExitStack

    import concourse.bass as bass
    import concourse.mybir as mybir
    import concourse.tile as tile
    from concourse import bacc
    from concourse.alu_op_type import AluOpType
    from concourse.masks import make_identity

    f32 = mybir.dt.float32
    i32 = mybir.dt.int32
    bf16 = mybir.dt.bfloat16
    fp16 = mybir.dt.float16
    AF = mybir.ActivationFunctionType

    P = 128
    F = F_OUT
    FE = F + 1                    # h columns + ones column
    K = K_IN
    assert nt % P == 0 and no % P == 0 and jw % P == 0 and nt % jw == 0
    ncj = nt // P                 # j chunks (permuted blocks)
    nw = nt // jw                 # windows
    cpw = jw // P                 # j chunks per window
    nic = no // P                 # i chunks (own rows)
    S = 512                       # i split for matmul N-dim / psum banks
    ns = no // S
    assert no % S == 0
    npg = nt // P                 # rows per partition in permuted layout (96)

    nc = bacc.Bacc("TRN2", target_bir_lowering=False, debug=False,
                   num_devices=1)

    inp = nc.dram_tensor("input", [nt, K], f32, kind="ExternalInput")
    inp_own = nc.dram_tensor("input_own", [no, K], f32, kind="ExternalInput")
    adjt = nc.dram_tensor("adjt", [nt, no], i32, kind="ExternalInput")
    w_d = nc.dram_tensor("W", [K, F], f32, kind="ExternalInput")
    a_d = nc.dram_tensor("a", [2 * F, 1], f32, kind="ExternalInput")
    out_d = nc.dram_tensor("out", [no, F], f32, kind="ExternalOutput")

    # permuted views: row (96p + n) -> [p, n]
    inp_r = inp.ap().rearrange("(p n) k -> p n k", p=P)
    adjt_r = adjt.ap().rearrange("(p n) i -> p n i", p=P)

    with tile.TileContext(nc) as tc, ExitStack() as ctx:
        consts = ctx.enter_context(tc.tile_pool(name="consts", bufs=1))

        identity = consts.tile([P, P], f32)
        make_identity(nc, identity)

        scr_ps = ctx.enter_context(
            tc.tile_pool(name="scr_ps", bufs=1, space="PSUM"))

        # ---- phase 0: Wa1 = W @ a1, Wa2 = W @ a2 -------------------------
        wwa2_sb = consts.tile([K, FE], f32)    # [W | Wa2] (128 x 65)
        nc.sync.dma_start(wwa2_sb[:, 0:F], w_d.ap())
        a_row = consts.tile([1, 2 * F], f32)   # a as a single-partition row
        nc.sync.dma_start(a_row[:], a_d.ap().rearrange("n o -> o n"))

        # full input, permuted chunks, 12 big contiguous DMAs
        t_sb = consts.tile([P, ncj, K], f32)
        gsz = 8
        for g in range(0, ncj, gsz):
            nc.sync.dma_start(t_sb[:, g:g + gsz, :], inp_r[:, g:g + gsz, :])

        ones_sb = consts.tile([P, P], f32)
        nc.vector.memset(ones_sb[:], 1.0)
        # replicate a across partitions via a K=1 matmul with a ones row
        a_rep = consts.tile([P, 2 * F], f32)
        a_rep_ps = scr_ps.tile([P, 2 * F], f32, tag="scr")
        nc.tensor.matmul(a_rep_ps[:], ones_sb[0:1, :], a_row[:],
                         start=True, stop=True)
        nc.vector.tensor_copy(a_rep[:], a_rep_ps[:])

        wa12_sb = consts.tile([K, 2], f32)
        wtmp = consts.tile([K, F], f32)
        nc.vector.tensor_tensor(wtmp[:], wwa2_sb[:, 0:F], a_rep[:, 0:F],
                                AluOpType.mult)
        nc.vector.tensor_reduce(wa12_sb[:, 0:1], wtmp[:],
                                mybir.AxisListType.X, AluOpType.add)
        nc.vector.tensor_tensor(wtmp[:], wwa2_sb[:, 0:F], a_rep[:, F:2 * F],
                                AluOpType.mult)
        nc.vector.tensor_reduce(wa12_sb[:, 1:2], wtmp[:],
                                mybir.AxisListType.X, AluOpType.add)
        nc.vector.tensor_copy(wwa2_sb[:, F:FE], wa12_sb[:, 1:2])
        wa1_rep = consts.tile([K, P], f32)     # Wa1 replicated to 128 cols
        nc.vector.tensor_scalar(wa1_rep[:], ones_sb[:], wa12_sb[:, 0:1], None,
                                AluOpType.mult)

        # ---- phase 1a: Wh1_rep[p, x] = Wh1[own x] for all p --------------
        # Wh1_rep = wa1_rep.T @ input_own.T ; input_own.T via PE transposes.
        wh1_rep = consts.tile([P, no], f32)
        wh1_h = consts.tile([P, no], fp16)     # fp16 copy for DVE leakyrelu
        into_sb = consts.tile([K, no], f32)    # input_own.T
        in_t = ctx.enter_context(tc.tile_pool(name="in_t", bufs=4))
        for ic in range(nic):
            ich = in_t.tile([P, K], f32, tag="ich")
            nc.sync.dma_start(ich[:], inp_own[ic * P:(ic + 1) * P, :])
            itp = scr_ps.tile([K, P], f32, tag="scr")
            nc.tensor.transpose(itp[:], ich[:], identity[:])
            nc.vector.tensor_copy(into_sb[:, ic * P:(ic + 1) * P], itp[:])
        for s in range(ns):
            w1p = scr_ps.tile([P, S], f32, tag="scr")
            nc.tensor.matmul(w1p[:], wa1_rep[:],
                             into_sb[:, s * S:(s + 1) * S],
                             start=True, stop=True)
            nc.vector.tensor_copy(wh1_rep[:, s * S:(s + 1) * S], w1p[:])
            nc.vector.tensor_copy(wh1_h[:, s * S:(s + 1) * S], w1p[:])

        # ---- phase 1b (emitted interleaved below): h_ext, Wh2 ------------
        h_ext = consts.tile([P, ncj, FE], bf16)
        wh2_sb = consts.tile([P, ncj], f32)
        nc.vector.memset(h_ext[:, :, F], 1.0)

        def phase1b_chunk(jc):
            # t_sb[:, jc, :].T via PE transpose; h_ext[:, jc, :] = [h | Wh2]
            jtp = scr_ps.tile([K, P], f32, tag="scr")
            nc.tensor.transpose(jtp[:], t_sb[:, jc, :], identity[:])
            jts = in_t.tile([K, P], f32, tag="jts")
            nc.vector.tensor_copy(jts[:], jtp[:])
            hw_ps = scr_ps.tile([P, FE], f32, tag="scr")
            nc.tensor.matmul(hw_ps[:], jts[:], wwa2_sb[:],
                             start=True, stop=True)
            nc.scalar.copy(h_ext[:, jc, 0:F], hw_ps[:, 0:F])
            nc.vector.tensor_copy(wh2_sb[:, jc:jc + 1], hw_ps[:, F:FE])

        # ---- phase 2: main loop over j windows / j chunks ----------------
        pt_pool = ctx.enter_context(
            tc.tile_pool(name="pt_acc", bufs=1, space="PSUM"))
        pt_ps = pt_pool.tile([FE, no], f32)

        n_act = int(round(act_frac * ncj))

        def lrelu_engine(jc):
            # deterministic interleave of ACT / DVE chunks
            return "act" if (jc * 7919) % ncj < n_act else "dve"

        def lrelu_chunk(jc, dst):
            if lrelu_engine(jc) == "act":
                nc.scalar.activation(dst, wh1_rep[:], AF.Prelu,
                                     bias=wh2_sb[:, jc:jc + 1],
                                     scale=1.0, alpha=ALPHA)
            else:
                # t = 0.2 * (Wh1 + Wh2) ; E = max(Wh1 + Wh2, t)  (4x DVE)
                nc.vector.tensor_scalar(dst, wh1_h[:],
                                        wh2_sb[:, jc:jc + 1], ALPHA,
                                        AluOpType.add, AluOpType.mult)
                nc.vector.scalar_tensor_tensor(
                    dst, wh1_h[:], wh2_sb[:, jc:jc + 1], dst,
                    AluOpType.add, AluOpType.max)

        # phase-1b chunks are produced spread over the first nw-1 windows,
        # always ahead of their first use by the main loop.
        per_w = -(-ncj // max(1, nw - 1))
        npair = cpw // 2
        assert cpw % 2 == 0

        with (
            tc.tile_pool(name="adjw", bufs=2) as adjw_pool,
            tc.tile_pool(name="tpool", bufs=3) as t_pool,
            tc.tile_pool(name="epool", bufs=2) as e_pool,
            tc.tile_pool(name="upool", bufs=3) as u_pool,
        ):
            for w in range(nw):
                # adj.T window DMA (SWDGE cast int32 -> bf16), one DMA
                adjw = adjw_pool.tile([P, cpw, no], bf16, tag="adjw",
                                      name=f"adjw_{w}")
                nc.gpsimd.dma_start(
                    adjw[:], adjt_r[:, w * cpw:(w + 1) * cpw, :])
                wchunks = list(range(w * per_w, min(ncj, (w + 1) * per_w)))
                for jp in range(npair):
                    for k, jc1b in enumerate(wchunks):
                        if k * npair // len(wchunks) == jp:
                            phase1b_chunk(jc1b)
                    jcs = [w * cpw + 2 * jp, w * cpw + 2 * jp + 1]
                    # leakyrelu both chunks (fp16), one batched exp -> bf16,
                    # one batched adjacency-mask multiply (bf16 2x DVE)
                    t_sb2 = t_pool.tile([P, 2, no], fp16, tag="t")
                    for q, jc in enumerate(jcs):
                        lrelu_chunk(jc, t_sb2[:, q, :])
                    e_sb = e_pool.tile([P, 2, no], bf16, tag="e")
                    nc.scalar.activation(e_sb[:], t_sb2[:], AF.Exp)
                    u_sb = u_pool.tile([P, 2, no], bf16, tag="u")
                    nc.vector.tensor_tensor(
                        u_sb[:], e_sb[:], adjw[:, 2 * jp:2 * jp + 2, :],
                        AluOpType.mult)
                    for q, jc in enumerate(jcs):
                        for s in range(ns):
                            nc.tensor.matmul(
                                pt_ps[:, s * S:(s + 1) * S],
                                h_ext[:, jc, :],
                                u_sb[:, q, s * S:(s + 1) * S],
                                start=(jc == 0),
                                stop=(jc == ncj - 1))

        # ---- phase 3: out = elu(P[:, :64] / P[:, 64]) --------------------
        pt_sb = consts.tile([FE, no], f32)
        nc.vector.tensor_copy(pt_sb[:], pt_ps[:])
        with tc.tile_pool(name="fin_sb", bufs=4) as fin_sb:
            for ic in range(nic):
                ptp = scr_ps.tile([P, FE], f32, tag="scr")
                nc.tensor.transpose(ptp[:], pt_sb[:, ic * P:(ic + 1) * P],
                                    identity[0:FE, 0:FE])
                rec = fin_sb.tile([P, 1], f32, tag="rec")
                nc.vector.reciprocal(rec[:], ptp[:, F:FE])
                hp = fin_sb.tile([P, F], f32, tag="hp")
                nc.vector.tensor_scalar(hp[:], ptp[:, 0:F], rec[:], None,
                                        AluOpType.mult)
                # elu(x) = max(x,0) + exp(min(x,0)) - 1
                mn = fin_sb.tile([P, F], f32, tag="mn")
                nc.vector.tensor_scalar(mn[:], hp[:], 0.0, None, AluOpType.min)
                nc.scalar.activation(mn[:], mn[:], AF.Exp)
                nc.vector.tensor_scalar(hp[:], hp[:], 0.0, None, AluOpType.max)
                ob = fin_sb.tile([P, F], f32, tag="ob")
                nc.vector.scalar_tensor_tensor(
                    ob[:], mn[:], 1.0, hp[:],
                    AluOpType.subtract, AluOpType.add)
                nc.sync.dma_start(out_d[ic * P:(ic + 1) * P, :], ob[:])

    nc.compile()
    return nc


_CACHE = {}


def _get_program(nt, no, jw, **kw):
    key = (nt, no, jw, tuple(sorted(kw.items())))
    if key not in _CACHE:
        _CACHE[key] = build_program(nt, no, jw, **kw)
    return _CACHE[key]


def make_in_maps(input, adj, W, a):
    input = np.ascontiguousarray(input, dtype=np.float32)
    adj = np.ascontiguousarray(adj, dtype=np.int32)
    W = np.ascontiguousarray(W, dtype=np.float32)
    a = np.ascontiguousarray(a, dtype=np.float32)
    nt = input.shape[0]
    no = nt // N_CORES
    in_maps = []
    for c in range(N_CORES):
        in_maps.append({
            "input": input,
            "input_own": np.ascontiguousarray(input[c * no:(c + 1) * no]),
            "adjt": np.ascontiguousarray(adj[c * no:(c + 1) * no].T),
            "W": W,
            "a": a,
        })
    return in_maps


def kernel(input, adj, W, a):
    from concourse.bass_utils import run_bass_kernel_spmd

    nt = input.shape[0]
    no = nt // N_CORES
    nc = _get_program(nt, no, 1024)
    in_maps = make_in_maps(input, adj, W, a)
    res = run_bass_kernel_spmd(nc, in_maps, list(range(N_CORES)))
    return np.concatenate([r["out"] for r in res.results], axis=0)


# revision 4
# speedup vs baseline: 1.5448x; 1.5349x over previous
"""Attention graph convolution (GAT layer) on 8 TRN2 NeuronCores.

Reference computation (all fp32):
    h   = input @ W                      # (N, 64)
    e   = leakyrelu(h@a1 + (h@a2).T)     # (N, N)
    att = softmax(where(adj>0, e, -inf)) # row softmax
    out = elu(att @ h)                   # (N, 64)

Sharding: rows of e/att (= output rows) are split across 8 cores,
1536 rows each.  h (N x 64) is computed on every core (tiny).

Host-side prep (layout only, byte counts unchanged): each core gets its
adjacency row-block TRANSPOSED (adjt[j, i] = adj[own_i, j]) and the
input transposed (inp_t = input.T).  The SWDGE cast DMA (int32 -> bf16,
exact for 0/1) then lands adj.T directly in the [j partitions, i free]
layout the PE contraction needs, and h is computed without any on-device
transposes.

Per-core algorithm (core owns rows I, |I| = 1536):
  - no max-subtraction softmax: |z| < ~30 so U = adj.T * exp(lrelu(z))
    cannot overflow in f32/bf16 and equals the reference numerator.
  - denominator via ones-column: P.T = [h | 1].T @ U.T;
    out = elu(P[:, :64] / P[:, 64]).
  - element-wise pipeline is all 2-byte SBUF tiles: leakyrelu z then
    max(z, 0.2z) via tensor_scalar (4x DVE) + scalar_tensor_tensor
    (DVE or GpSimd, schedule-balanced) or Prelu on ACT; exp on ACT
    (fp16 -> bf16); adjacency mask on DVE (bf16, 2x mode).
  - accumulation matmuls are bf16 (1 cycle/row vs fp32's 4).
"""

import numpy as np

N_TOTAL = 12288
K_IN = 128
F_OUT = 64
N_CORES = 8
ALPHA = 0.2


def build_program(
    nt: int,          # total nodes (rows of adjt)
    no: int,          # nodes owned by this core (cols of adjt)
    jw: int,          # j window size (adj.T rows resident in SBUF at once)
    act_frac: float = 0.3,   # fraction of j chunks with leakyrelu on ACT
    gp_frac: float = 0.0,    # fraction of chunks whose 2nd lrelu op on GpSimd
):
    from contextlib import ExitStack

    import concourse.bass as bass
    import concourse.mybir as mybir
    import concourse.tile as tile
    from concourse import bacc
    from concourse.alu_op_type import AluOpType

    f32 = mybir.dt.float32
    i32 = mybir.dt.int32
    bf16 = mybir.dt.bfloat16
    fp16 = mybir.dt.float16
    AF = mybir.ActivationFunctionType

    P = 128
    F = F_OUT
    FE = F + 1                    # h columns + ones column
    K = K_IN
    assert nt % P == 0 and no % P == 0 and jw % P == 0 and nt % jw == 0
    ncj = nt // P                 # j chunks
    nw = nt // jw                 # windows
    cpw = jw // P                 # j chunks per window
    nic = no // P                 # i chunks (own rows)
    S = 512                       # i split for matmul N-dim / psum banks
    ns = no // S
    assert no % S == 0

    nc = bacc.Bacc("TRN2", target_bir_lowering=False, debug=False,
                   num_devices=1)

    inp_t = nc.dram_tensor("inp_t", [K, nt], f32, kind="ExternalInput")
    inp_own_t = nc.dram_tensor("inp_own_t", [K, no], f32,
                               kind="ExternalInput")
    adjt = nc.dram_tensor("adjt", [nt, no], i32, kind="ExternalInput")
    w_d = nc.dram_tensor("W", [K, F], f32, kind="ExternalInput")
    a_d = nc.dram_tensor("a", [2 * F, 1], f32, kind="ExternalInput")
    out_d = nc.dram_tensor("out", [no, F], f32, kind="ExternalOutput")

    # adj.T rows as [partition, chunk, i]: row (n*128 + p) -> [p, n]
    adjt_r = adjt.ap().rearrange("(n p) i -> p n i", p=P)

    with tile.TileContext(nc) as tc, ExitStack() as ctx:
        consts = ctx.enter_context(tc.tile_pool(name="consts", bufs=1))
        scr_ps = ctx.enter_context(
            tc.tile_pool(name="scr_ps", bufs=1, space="PSUM"))
        p1b_ps = ctx.enter_context(
            tc.tile_pool(name="p1b_ps", bufs=2, space="PSUM"))
        adjw_pool = ctx.enter_context(tc.tile_pool(name="adjw", bufs=2))

        # ---- adj.T window 0 DMA first: the SWDGE stream is the critical
        # resource; get it going before anything else.
        adjw_tiles = {}

        def issue_adjw(w):
            t = adjw_pool.tile([P, cpw, no], bf16, tag="adjw",
                               name=f"adjw_{w}")
            nc.gpsimd.dma_start(t[:], adjt_r[:, w * cpw:(w + 1) * cpw, :])
            adjw_tiles[w] = t

        issue_adjw(0)
        issue_adjw(1)

        # ---- phase 0: Wa1 = W @ a1, Wa2 = W @ a2 -------------------------
        wwa2_sb = consts.tile([K, FE], f32)    # [W | Wa2] (128 x 65)
        nc.sync.dma_start(wwa2_sb[:, 0:F], w_d.ap())
        a_row = consts.tile([1, 2 * F], f32)   # a as a single-partition row
        nc.sync.dma_start(a_row[:], a_d.ap().rearrange("n o -> o n"))
        io_t = consts.tile([K, no], f32)       # input_own.T
        nc.sync.dma_start(io_t[:], inp_own_t.ap())

        # full input.T, 12 sub-DMAs so phase-1b chunks unblock early
        t_sb = consts.tile([K, nt], f32)
        gsz = 1024
        for g in range(0, nt, gsz):
            nc.sync.dma_start(t_sb[:, g:g + gsz], inp_t.ap()[:, g:g + gsz])

        ones_sb = consts.tile([P, P], f32)
        nc.vector.memset(ones_sb[:], 1.0)
        # replicate a across partitions via a K=1 matmul with a ones row
        a_rep = consts.tile([P, 2 * F], f32)
        a_rep_ps = scr_ps.tile([P, 2 * F], f32, tag="scr")
        nc.tensor.matmul(a_rep_ps[:], ones_sb[0:1, :], a_row[:],
                         start=True, stop=True)
        nc.vector.tensor_copy(a_rep[:], a_rep_ps[:])

        wa12_sb = consts.tile([K, 2], f32)
        wtmp = consts.tile([K, F], f32)
        nc.vector.tensor_tensor(wtmp[:], wwa2_sb[:, 0:F], a_rep[:, 0:F],
                                AluOpType.mult)
        nc.vector.tensor_reduce(wa12_sb[:, 0:1], wtmp[:],
                                mybir.AxisListType.X, AluOpType.add)
        nc.vector.tensor_tensor(wtmp[:], wwa2_sb[:, 0:F], a_rep[:, F:2 * F],
                                AluOpType.mult)
        nc.vector.tensor_reduce(wa12_sb[:, 1:2], wtmp[:],
                                mybir.AxisListType.X, AluOpType.add)
        nc.vector.tensor_copy(wwa2_sb[:, F:FE], wa12_sb[:, 1:2])
        wa1_rep = consts.tile([K, P], f32)     # Wa1 replicated to 128 cols
        nc.vector.tensor_scalar(wa1_rep[:], ones_sb[:], wa12_sb[:, 0:1], None,
                                AluOpType.mult)

        # ---- phase 1a: Wh1_rep[p, x] = Wh1[own x] for all p --------------
        wh1_rep = consts.tile([P, no], f32)
        wh1_h = consts.tile([P, no], fp16)     # fp16 copy for DVE leakyrelu
        for s in range(ns):
            w1p = scr_ps.tile([P, S], f32, tag="scr")
            nc.tensor.matmul(w1p[:], wa1_rep[:], io_t[:, s * S:(s + 1) * S],
                             start=True, stop=True)
            nc.vector.tensor_copy(wh1_rep[:, s * S:(s + 1) * S], w1p[:])
            nc.vector.tensor_copy(wh1_h[:, s * S:(s + 1) * S], w1p[:])

        # ---- phase 1b (emitted interleaved below): h_ext, Wh2 ------------
        h_ext = consts.tile([P, ncj, FE], bf16)
        wh2_sb = consts.tile([P, ncj], f32)
        nc.vector.memset(h_ext[:, :, F], 1.0)

        def phase1b_chunk(jc):
            # h_ext[:, jc, :] = input[chunk jc] @ [W | Wa2], no transposes:
            # t_sb chunk is already input.T
            hw_ps = p1b_ps.tile([P, FE], f32, tag="p1b")
            nc.tensor.matmul(hw_ps[:], t_sb[:, jc * P:(jc + 1) * P],
                             wwa2_sb[:], start=True, stop=True)
            nc.scalar.copy(h_ext[:, jc, 0:F], hw_ps[:, 0:F])
            nc.vector.tensor_copy(wh2_sb[:, jc:jc + 1], hw_ps[:, F:FE])

        # ---- phase 2: main loop over j windows / j chunks ----------------
        pt_pool = ctx.enter_context(
            tc.tile_pool(name="pt_acc", bufs=1, space="PSUM"))
        pt_ps = pt_pool.tile([FE, no], f32)

        n_act = int(round(act_frac * ncj))
        n_gp = int(round(gp_frac * ncj))

        def lrelu_engine(jc):
            # deterministic interleave of ACT / DVE / DVE+GpSimd chunks
            r = (jc * 7919) % ncj
            if r < n_act:
                return "act"
            if r < n_act + n_gp:
                return "gp"
            return "dve"

        def lrelu_chunk(jc, dst):
            eng = lrelu_engine(jc)
            if eng == "act":
                nc.scalar.activation(dst, wh1_rep[:], AF.Prelu,
                                     bias=wh2_sb[:, jc:jc + 1],
                                     scale=1.0, alpha=ALPHA)
                return
            # t = 0.2 * (Wh1 + Wh2) ; E = max(Wh1 + Wh2, t)
            nc.vector.tensor_scalar(dst, wh1_h[:],
                                    wh2_sb[:, jc:jc + 1], ALPHA,
                                    AluOpType.add, AluOpType.mult)
            eng2 = nc.gpsimd if eng == "gp" else nc.vector
            eng2.scalar_tensor_tensor(
                dst, wh1_h[:], wh2_sb[:, jc:jc + 1], dst,
                AluOpType.add, AluOpType.max)

        # phase-1b chunks are produced spread over the first nw-1 windows,
        # always ahead of their first use by the main loop.
        per_w = -(-ncj // max(1, nw - 1))
        npair = cpw // 2
        assert cpw % 2 == 0

        with (
            tc.tile_pool(name="tpool", bufs=3) as t_pool,
            tc.tile_pool(name="epool", bufs=2) as e_pool,
            tc.tile_pool(name="upool", bufs=3) as u_pool,
        ):
            for w in range(nw):
                if w + 2 < nw:
                    issue_adjw(w + 2)
                adjw = adjw_tiles.pop(w)
                wchunks = list(range(w * per_w, min(ncj, (w + 1) * per_w)))
                for jp in range(npair):
                    for k, jc1b in enumerate(wchunks):
                        if k * npair // len(wchunks) == jp:
                            phase1b_chunk(jc1b)
                    jcs = [w * cpw + 2 * jp, w * cpw + 2 * jp + 1]
                    # leakyrelu both chunks (fp16), one batched exp -> bf16,
                    # one batched adjacency-mask multiply (bf16 2x DVE)
                    t_sb2 = t_pool.tile([P, 2, no], fp16, tag="t")
                    for q, jc in enumerate(jcs):
                        lrelu_chunk(jc, t_sb2[:, q, :])
                    e_sb = e_pool.tile([P, 2, no], bf16, tag="e")
                    nc.scalar.activation(e_sb[:], t_sb2[:], AF.Exp)
                    u_sb = u_pool.tile([P, 2, no], bf16, tag="u")
                    nc.vector.tensor_tensor(
                        u_sb[:], e_sb[:], adjw[:, 2 * jp:2 * jp + 2, :],
                        AluOpType.mult)
                    for q, jc in enumerate(jcs):
                        for s in range(ns):
                            nc.tensor.matmul(
                                pt_ps[:, s * S:(s + 1) * S],
                                h_ext[:, jc, :],
                                u_sb[:, q, s * S:(s + 1) * S],
                                start=(jc == 0),
                                stop=(jc == ncj - 1))

        # ---- phase 3: out = elu(P[:, :64] / P[:, 64]) --------------------
        identity = consts.tile([P, P], f32)
        from concourse.masks import make_identity
        make_identity(nc, identity)
        pt_sb = consts.tile([FE, no], f32)
        nc.vector.tensor_copy(pt_sb[:], pt_ps[:])
        with tc.tile_pool(name="fin_sb", bufs=4) as fin_sb:
            for ic in range(nic):
                ptp = scr_ps.tile([P, FE], f32, tag="scr")
                nc.tensor.transpose(ptp[:], pt_sb[:, ic * P:(ic + 1) * P],
                                    identity[0:FE, 0:FE])
                rec = fin_sb.tile([P, 1], f32, tag="rec")
                nc.vector.reciprocal(rec[:], ptp[:, F:FE])
                hp = fin_sb.tile([P, F], f32, tag="hp")
                nc.vector.tensor_scalar(hp[:], ptp[:, 0:F], rec[:], None,
                                        AluOpType.mult)
                # elu(x) = max(x,0) + exp(min(x,0)) - 1
                mn = fin_sb.tile([P, F], f32, tag="mn")
                nc.vector.tensor_scalar(mn[:], hp[:], 0.0, None, AluOpType.min)
                nc.scalar.activation(mn[:], mn[:], AF.Exp)
                nc.vector.tensor_scalar(hp[:], hp[:], 0.0, None, AluOpType.max)
                ob = fin_sb.tile([P, F], f32, tag="ob")
                nc.vector.scalar_tensor_tensor(
                    ob[:], mn[:], 1.0, hp[:],
                    AluOpType.subtract, AluOpType.add)
                nc.sync.dma_start(out_d[ic * P:(ic + 1) * P, :], ob[:])

    nc.compile()
    return nc


_CACHE = {}


def _get_program(nt, no, jw, **kw):
    key = (nt, no, jw, tuple(sorted(kw.items())))
    if key not in _CACHE:
        _CACHE[key] = build_program(nt, no, jw, **kw)
    return _CACHE[key]


def make_in_maps(input, adj, W, a):
    input = np.ascontiguousarray(input, dtype=np.float32)
    adj = np.ascontiguousarray(adj, dtype=np.int32)
    W = np.ascontiguousarray(W, dtype=np.float32)
    a = np.ascontiguousarray(a, dtype=np.float32)
    nt = input.shape[0]
    no = nt // N_CORES
    inp_t = np.ascontiguousarray(input.T)
    in_maps = []
    for c in range(N_CORES):
        in_maps.append({
            "inp_t": inp_t,
            "inp_own_t": np.ascontiguousarray(inp_t[:, c * no:(c + 1) * no]),
            "adjt": np.ascontiguousarray(adj[c * no:(c + 1) * no].T),
            "W": W,
            "a": a,
        })
    return in_maps


def kernel(input, adj, W, a):
    from concourse.bass_utils import run_bass_kernel_spmd

    nt = input.shape[0]
    no = nt // N_CORES
    nc = _get_program(nt, no, 1024)
    in_maps = make_in_maps(input, adj, W, a)
    res = run_bass_kernel_spmd(nc, in_maps, list(range(N_CORES)))
    return np.concatenate([r["out"] for r in res.results], axis=0)
